# revision 36
# baseline (speedup 1.0000x reference)
"""Trainium2 Bass kernel for the combined loss (KL + CE + InfoNCE + focal + adv CE).

Strategy (8 NeuronCores, symmetric half-band InfoNCE):
  - o / master / o_adv sharded by rows (512/core), shipped bf16, packed as one
    [128, 3000] tile per 128-row group (one DMA each).
  - InfoNCE: feats = concat(feat_pooled, feat_pooled_masked) -> [8192, 256],
    transposed to [256, 8192] bf16 with a per-core GLOBAL column roll (core's
    1024 query rows land at rolled block positions {0..3, 32..35}).  Exploits
    G = G^T: each query block computes only its contiguous 33-block extent
    [self, +1..+32] of the circulant band, with triangular masks on the self
    and +32 blocks so every unordered pair is computed exactly once
    fleet-wide.  Each exp(l_ij) serves row i via the ScalarE accumulate (row
    sums) and row j via column sums: the 8 query blocks' exp tiles are first
    accumulated into one [128, 8192] rolled-column tile on the DVE, then a
    single pass of 64 rank-reducing matmuls (exp block as lhsT x ones column)
    extracts all block column sums.  Per-core partial row/col sums + positive
    trace go to the host, which assembles the 8192 row sums, takes the log,
    and averages (the only cross-core reduction).
  - Normalization: squares on DVE, column sums-of-squares via a bf16
    ones-matmul, rn = exp(-0.5*ln(s)) on ScalarE, normalize mult on DVE.
  - The activation-table map is restricted so Exp and Ln both resolve to
    natural_log_exp_and_others: one table load, no Ln<->Exp thrashing.
"""

import numpy as np
import ml_dtypes

import concourse.bacc as bacc
import concourse.tile as tile
from concourse import mybir
from concourse.bass_utils import run_bass_kernel_spmd

F32 = mybir.dt.float32
BF16 = mybir.dt.bfloat16
FP16 = mybir.dt.float16
F8 = mybir.dt.float8e4
AF = mybir.ActivationFunctionType
ALU = mybir.AluOpType
AX = mybir.AxisListType

NCORES = 8
B, C, D = 4096, 1000, 256
RB = B // NCORES          # 512 rows of the [B, C] tensors per core
NT = RB // 128            # 4 row-tiles per core
N2 = 2 * B                # 8192 infoNCE rows
NBLK = N2 // 128          # 64 column blocks
QBLKS = [0, 1, 2, 3, 32, 33, 34, 35]   # rolled block positions of queries
NQ = len(QBLKS)
ESW = 33 * 128            # extent: self block + 32 band blocks = 4224 cols
KCH = 2048                # gram / psum chunk width
SUB = 512                 # matmul moving free dim
NSLOT = 3                 # rowsum slots per query block (2 chunks + tail)

KL_TEMP = 4.0
KL_INTERP = 0.5
NCE_TEMP = 0.07
NEG_BIG = -1.0e9

_orig_gat = bacc.get_activation_tables


def _pinned_tables(arch):
    """Keep canonical set order/ids but make Exp and Ln resolve only to
    natural_log_exp_and_others so the planner never reloads tables."""
    t = _orig_gat(arch)
    if "natural_log_exp_and_others" in t:
        nle = t["natural_log_exp_and_others"]
        if AF.Exp in nle and AF.Ln in nle:
            for name, s in t.items():
                if name != "natural_log_exp_and_others":
                    s.discard(AF.Exp)
                    s.discard(AF.Ln)
    return t


def _col_runs(start, e0, e1):
    """Contiguous rolled-column runs covering extent offsets [e0, e1)."""
    runs = []
    e = e0
    while e < e1:
        rs = (start + e) % N2
        ln = min(e1 - e, N2 - rs)
        runs.append((rs, e, ln))
        e += ln
    return runs


def _build_module():
    bacc.get_activation_tables = _pinned_tables
    nc = bacc.Bacc("TRN2", target_bir_lowering=False, debug=False)

    oma_d = nc.dram_tensor("oma", [RB, 3 * C], BF16, kind="ExternalInput")
    ft_d = nc.dram_tensor("ft", [256, N2], BF16, kind="ExternalInput")
    fp_d = nc.dram_tensor("fp", [128, C + 8], FP16, kind="ExternalInput")
    res_d = nc.dram_tensor("res", [8, 1], F32, kind="ExternalOutput")
    rsp_d = nc.dram_tensor("rsp", [128, NQ * NSLOT], F32,
                           kind="ExternalOutput")
    # csp[p, blk] = partial row sum for rolled row 128*blk+p
    csp_d = nc.dram_tensor("csp", [128, NBLK], F32, kind="ExternalOutput")

    r_idx = np.arange(128)[:, None]
    c_idx = np.arange(128)[None, :]
    # matmul adds lhsT^T; bake the transpose into the constants.
    cb16 = np.concatenate([
        np.eye(128),                          # identb
        np.ones((128, 128)),                  # onesb
        NEG_BIG * (r_idx > c_idx),            # keep c<=r   (strict mask^T)
        NEG_BIG * (r_idx >= c_idx),           # keep c<r    (incl mask^T)
    ], axis=1).astype(ml_dtypes.bfloat16)
    cf32 = np.concatenate([np.eye(128), np.ones((128, 1))],
                          axis=1).astype(np.float32)
    cb16_d = nc.inline_tensor(cb16, "cb16")
    cf32_d = nc.inline_tensor(cf32, "cf32")

    from contextlib import ExitStack
    with tile.TileContext(nc) as tc:
        with (
            tc.tile_pool(name="persist", bufs=1) as persist,
            tc.tile_pool(name="io", bufs=2) as iop,
            tc.tile_pool(name="scr", bufs=4) as scrp,
            tc.tile_pool(name="vec", bufs=1) as vecp,
            tc.tile_pool(name="ps", bufs=2, space="PSUM") as psp,
            ExitStack() as late_ctx,
        ):
            dma = nc.default_dma_engine.dma_start

            cb_t = persist.tile([128, 512], BF16, tag="cb16")
            dma(out=cb_t[:], in_=cb16_d[:])
            identb_t = cb_t[:, 0:128]
            onesb_t = cb_t[:, 128:256]
            masks_t = cb_t[:, 256:384]
            maski_t = cb_t[:, 384:512]
            cf_t = persist.tile([128, 129], F32, tag="cf32")
            dma(out=cf_t[:], in_=cf32_d[:])
            ident_t = cf_t[:, 0:128]
            onesf_t = cf_t[:, 128:129]
            fp_t = persist.tile([128, C + 8], FP16, tag="fp")
            dma(out=fp_t[:], in_=fp_d[:])
            iota_t = fp_t[:, 0:C]
            tg_t = fp_t[:, C:C + 4]
            ta_t = fp_t[:, C + 4:C + 8]

            # normalized features, fp8, both contraction k-tiles in one 3D
            # tile so DoubleRow matmuls fuse the full 256-deep contraction
            hnq = persist.tile([128, 2, N2], F8, tag="hnq")
            essum = persist.tile([128, N2], BF16, tag="essum")
            rsp_t = persist.tile([128, NQ * NSLOT], F32, tag="rsp")
            trace_t = vecp.tile([128, 4], F32, tag="trace")

            # cekl per-row stat slots; the four lse stats share one tile so a
            # single Ln covers them
            stats = vecp.tile([128, 16], F32, tag="stats")
            PP = vecp.tile([128, NT], F32, tag="PP")
            GO = vecp.tile([128, NT], F32, tag="GO")
            GA = vecp.tile([128, NT], F32, tag="GA")

            oma_tiles = {}

            def cekl_dma(t):
                rsl = slice(t * 128, (t + 1) * 128)
                oma_t = iop.tile([128, 3 * C], BF16, tag="oma")
                dma(out=oma_t[:], in_=oma_d[rsl, :])
                oma_tiles[t] = oma_t

            def cekl_tile(t):
                oma_t = oma_tiles[t]
                o_t = oma_t[:, 0:C]
                m_t = oma_t[:, C:2 * C]
                a_t = oma_t[:, 2 * C:3 * C]

                e1 = scrp.tile([128, C], BF16, tag="scr1000")
                nc.scalar.activation(e1[:], o_t, AF.Exp, scale=1.0,
                                     accum_out=stats[:, t:t + 1])
                e2 = scrp.tile([128, C], BF16, tag="scr1000")
                nc.scalar.activation(e2[:], o_t, AF.Exp,
                                     scale=float(1.0 / KL_TEMP),
                                     accum_out=stats[:, 4 + t:5 + t])
                em_t = iop.tile([128, C], BF16, tag="em")
                nc.scalar.activation(em_t[:], m_t, AF.Exp,
                                     scale=float(1.0 / KL_TEMP),
                                     accum_out=stats[:, 8 + t:9 + t])
                e3 = scrp.tile([128, C], BF16, tag="scr1000")
                nc.scalar.activation(e3[:], a_t, AF.Exp, scale=1.0,
                                     accum_out=stats[:, 12 + t:13 + t])

                d_t = iop.tile([128, C], BF16, tag="d")
                nc.vector.tensor_sub(d_t[:], m_t, o_t)
                pr = scrp.tile([128, C], BF16, tag="scr1000")
                nc.vector.scalar_tensor_tensor(
                    out=pr[:], in0=d_t[:], scalar=1.0, in1=em_t[:],
                    op0=ALU.mult, op1=ALU.mult, accum_out=PP[:, t:t + 1])
                g1 = scrp.tile([128, C], BF16, tag="scr1000")
                nc.vector.scalar_tensor_tensor(
                    out=g1[:], in0=iota_t, scalar=tg_t[:, t:t + 1],
                    in1=o_t, op0=ALU.is_equal, op1=ALU.mult,
                    accum_out=GO[:, t:t + 1])
                g2 = scrp.tile([128, C], BF16, tag="scr1000")
                nc.vector.scalar_tensor_tensor(
                    out=g2[:], in0=iota_t, scalar=ta_t[:, t:t + 1],
                    in1=a_t, op0=ALU.is_equal, op1=ALU.mult,
                    accum_out=GA[:, t:t + 1])

            # ---- DMA order: first two cekl tiles lead, then consts (already
            # queued), then the remaining cekl tiles, then the features
            cekl_dma(0)
            cekl_dma(1)
            cekl_dma(2)
            cekl_dma(3)

            cekl_tile(0)
            cekl_tile(1)

            raw_pool_cm = tc.tile_pool(name="raw", bufs=1)
            rawp = raw_pool_cm.__enter__()
            h0 = rawp.tile([128, N2], BF16, tag="h0")
            h1 = rawp.tile([128, N2], BF16, tag="h1")
            sq0 = rawp.tile([128, N2], BF16, tag="sq0")
            sq1 = rawp.tile([128, N2], BF16, tag="sq1")
            rn = rawp.tile([128, N2], BF16, tag="rn")
            dma(out=h0[:], in_=ft_d[0:128, :])
            dma(out=h1[:], in_=ft_d[128:256, :])

            # ---- normalization pipeline, chunked
            NCH = N2 // KCH
            for ch in range(NCH):
                sl = slice(ch * KCH, (ch + 1) * KCH)
                nc.vector.tensor_mul(sq0[:, sl], h0[:, sl], h0[:, sl])
                nc.vector.tensor_mul(sq1[:, sl], h1[:, sl], h1[:, sl])
                ps_n = psp.tile([128, KCH], F32, tag="ps")
                for half, sq in ((0, sq0), (1, sq1)):
                    for s in range(KCH // SUB):
                        c0 = ch * KCH + s * SUB
                        osl = slice(s * SUB, (s + 1) * SUB)
                        nc.tensor.matmul(ps_n[:, osl], onesb_t,
                                         sq[:, c0:c0 + SUB],
                                         start=(half == 0), stop=(half == 1))
                nc.scalar.activation(rn[:, sl], ps_n[:], AF.Ln)
                nc.scalar.activation(rn[:, sl], rn[:, sl], AF.Exp, scale=-0.5)
                nc.vector.tensor_mul(hnq[:, 0, sl], h0[:, sl], rn[:, sl])
                nc.vector.tensor_mul(hnq[:, 1, sl], h1[:, sl], rn[:, sl])

            cekl_tile(2)
            cekl_tile(3)

            # raw features / rn are dead; release before the es pool opens.
            raw_pool_cm.__exit__(None, None, None)
            esp = late_ctx.enter_context(tc.tile_pool(name="es", bufs=3))

            # ---- symmetric half-band InfoNCE ----
            CHUNKS = [(0, KCH), (KCH, KCH), (2 * KCH, 128)]
            # ct wave 1: blocks untouched by the second-half query extents
            wave1 = [b for b in range(NBLK)
                     if not any((b - Bq) % NBLK <= 32 for Bq in QBLKS[4:])]

            def ct_extract(blocks):
                # shares the "ps" psum slots with the gram chunks
                ct_ps = psp.tile([128, len(blocks)], F32, tag="ps")
                for i, bk in enumerate(blocks):
                    nc.tensor.matmul(ct_ps[:, i:i + 1],
                                     essum[:, bk * 128:(bk + 1) * 128],
                                     onesb_t[:, 0:1], start=True, stop=True)
                ct_sb = vecp.tile([128, len(blocks)], F32,
                                  tag=f"ct_sb{blocks[0]}")
                nc.scalar.copy(ct_sb[:], ct_ps[:])
                i = 0
                while i < len(blocks):
                    j = i
                    while j + 1 < len(blocks) and blocks[j + 1] == blocks[j] + 1:
                        j += 1
                    dma(out=csp_d[:, blocks[i]:blocks[j] + 1],
                        in_=ct_sb[:, i:j + 1])
                    i = j + 1

            DR = mybir.MatmulPerfMode.DoubleRow
            written = np.zeros(NBLK, dtype=bool)

            def essum_chunk(es_t, cb, base0, cw):
                # accumulate this chunk of the exp tile into the rolled-column
                # accumulator; first touch of a block is a copy (no memset)
                b0, b1 = base0 // 128, (base0 + cw) // 128
                ext = [( (cb // 128) + k) % NBLK for k in range(b0, b1)]
                seg = 0
                while seg < len(ext):
                    new = not written[ext[seg]]
                    end = seg
                    while end + 1 < len(ext) and \
                            (not written[ext[end + 1]]) == new:
                        end += 1
                    for (rs_c, e, ln) in _col_runs(
                            cb, base0 + seg * 128, base0 + (end + 1) * 128):
                        if new:
                            nc.vector.tensor_copy(essum[:, rs_c:rs_c + ln],
                                                  es_t[:, e:e + ln])
                        else:
                            nc.vector.tensor_add(essum[:, rs_c:rs_c + ln],
                                                 essum[:, rs_c:rs_c + ln],
                                                 es_t[:, e:e + ln])
                    for b in ext[seg:end + 1]:
                        written[b] = True
                    seg = end + 1

            for qi, Bq in enumerate(QBLKS):
                cb = 128 * Bq
                lhsT = hnq[:, 0:2, cb:cb + 128]
                es_t = esp.tile([128, ESW], BF16, tag="es")
                for ci, (base0, cw) in enumerate(CHUNKS):
                    ps_k = psp.tile([128, cw], F32, tag="ps")
                    for s in range((cw + SUB - 1) // SUB):
                        b0 = base0 + s * SUB
                        b1 = min(base0 + cw, b0 + SUB)
                        for (rs_c, e, ln) in _col_runs(cb, b0, b1):
                            off = e - base0
                            nc.tensor.matmul(
                                ps_k[:, off:off + ln], lhsT,
                                hnq[:, 0:2, rs_c:rs_c + ln],
                                start=True, stop=True, perf_mode=DR)
                    if ci == 0:
                        # self block: keep strict lower triangle
                        # (post-group accumulate into the closed psum region)
                        nc.tensor.matmul(ps_k[:, 0:128], maski_t,
                                         identb_t, start=False, stop=True,
                                         skip_group_check=True)
                    if ci == 2:
                        # +32 block: tie-break mask on the pair diagonal
                        mk = masks_t if qi < 4 else maski_t
                        nc.tensor.matmul(ps_k[:, 0:128], mk,
                                         identb_t, start=False, stop=True,
                                         skip_group_check=True)
                    nc.scalar.activation(
                        es_t[:, base0:base0 + cw], ps_k[:], AF.Exp,
                        scale=float(1.0 / NCE_TEMP),
                        accum_out=rsp_t[:, qi * NSLOT + ci:qi * NSLOT + ci + 1])
                    if ci == 2 and qi < 4:
                        tr_scr = scrp.tile([128, 128], F32, tag="trscr")
                        nc.vector.scalar_tensor_tensor(
                            out=tr_scr[:], in0=ps_k[:, 0:128], scalar=1.0,
                            in1=ident_t, op0=ALU.mult, op1=ALU.mult,
                            accum_out=trace_t[:, qi:qi + 1])
                    essum_chunk(es_t, cb, base0, cw)
                if qi == 3:
                    ct_extract(wave1)

            ct_extract([b for b in range(NBLK) if b not in wave1])

            # ---- epilogue on [128, NT] stat vectors ----
            lse_all = vecp.tile([128, 16], F32, tag="lse_all")
            nc.scalar.activation(lse_all[:], stats[:], AF.Ln)
            lse1 = lse_all[:, 0:4]
            lseT = lse_all[:, 4:8]
            lsem = lse_all[:, 8:12]
            lsea = lse_all[:, 12:16]

            ce = vecp.tile([128, NT], F32, tag="ce")
            nc.vector.tensor_sub(ce[:], lse1, GO[:])
            adv = vecp.tile([128, NT], F32, tag="adv")
            nc.vector.tensor_sub(adv[:], lsea, GA[:])

            # kl_row = PP/(T*SM) - lsem + lseT
            invSM = vecp.tile([128, NT], F32, tag="invSM")
            nc.vector.reciprocal(invSM[:], stats[:, 8:12])
            kl = vecp.tile([128, NT], F32, tag="kl")
            nc.vector.tensor_mul(kl[:], PP[:], invSM[:])
            nc.vector.tensor_scalar_mul(kl[:], kl[:], float(1.0 / KL_TEMP))
            nc.vector.tensor_sub(kl[:], kl[:], lsem)
            nc.vector.tensor_add(kl[:], kl[:], lseT)

            # focal_row = (1-pt)^gamma * ce,  pt = exp(-ce)
            pt = vecp.tile([128, NT], F32, tag="pt")
            nc.scalar.activation(pt[:], ce[:], AF.Exp, scale=-1.0)
            c1 = vecp.tile([128, NT], F32, tag="c1")
            nc.vector.tensor_scalar(c1[:], pt[:], 0.5, None, op0=ALU.is_lt)
            c2 = vecp.tile([128, NT], F32, tag="c2")
            nc.vector.tensor_scalar(c2[:], pt[:], 0.2, None, op0=ALU.is_lt)
            # w = (1-pt)^gamma with gamma = 1 + 2*c1 + 2*c2:
            #   w = u * (u^2)^c1 * (u^2)^c2,  (u^2)^ci = 1 + ci*(u^2-1)
            u = vecp.tile([128, NT], F32, tag="u")
            nc.vector.tensor_scalar(u[:], pt[:], -1.0, 1.0,
                                    op0=ALU.mult, op1=ALU.add)
            t2 = vecp.tile([128, NT], F32, tag="t2")
            nc.vector.tensor_mul(t2[:], u[:], u[:])
            nc.vector.tensor_scalar(t2[:], t2[:], -1.0, None, op0=ALU.add)
            f1 = vecp.tile([128, NT], F32, tag="f1")
            nc.vector.tensor_mul(f1[:], c1[:], t2[:])
            nc.vector.tensor_scalar(f1[:], f1[:], 1.0, None, op0=ALU.add)
            f2 = vecp.tile([128, NT], F32, tag="f2")
            nc.vector.tensor_mul(f2[:], c2[:], t2[:])
            nc.vector.tensor_scalar(f2[:], f2[:], 1.0, None, op0=ALU.add)
            w = vecp.tile([128, NT], F32, tag="w")
            nc.vector.tensor_mul(w[:], u[:], f1[:])
            nc.vector.tensor_mul(w[:], w[:], f2[:])
            foc = vecp.tile([128, NT], F32, tag="foc")
            nc.vector.tensor_mul(foc[:], w[:], ce[:])

            # ---- reduce to partial sums, then across partitions via PE ----
            acc = vecp.tile([128, 8], F32, tag="acc")
            nc.vector.reduce_sum(acc[:, 0:1], kl[:], axis=AX.X)
            nc.vector.reduce_sum(acc[:, 1:2], ce[:], axis=AX.X)
            nc.vector.reduce_sum(acc[:, 2:3], adv[:], axis=AX.X)
            nc.vector.reduce_sum(acc[:, 3:4], foc[:], axis=AX.X)
            nc.vector.reduce_sum(acc[:, 4:5], trace_t[:], axis=AX.X)
            nc.vector.memset(acc[:, 5:8], 0.0)

            ps_f = psp.tile([8, 1], F32, tag="ps")
            nc.tensor.matmul(ps_f[:], acc[:], onesf_t,
                             start=True, stop=True)
            out_sb = vecp.tile([8, 1], F32, tag="out_sb")
            nc.scalar.copy(out_sb[:], ps_f[:])
            dma(out=res_d[:], in_=out_sb[:])
            dma(out=rsp_d[:], in_=rsp_t[:])

    nc.compile()
    return nc


_NC = None


def _get_nc():
    global _NC
    if _NC is None:
        _NC = _build_module()
    return _NC


def _prep_inputs(output, target, master_net_pred, feat_pooled,
                 feat_pooled_masked, output_adv, target_adv):
    o = np.asarray(output, dtype=np.float32)
    m = np.asarray(master_net_pred, dtype=np.float32)
    a = np.asarray(output_adv, dtype=np.float32)
    oma = np.concatenate([o.reshape(B // 128, 128, C),
                          m.reshape(B // 128, 128, C),
                          a.reshape(B // 128, 128, C)],
                         axis=2).astype(ml_dtypes.bfloat16)  # [B/128,128,3C]
    tg = np.asarray(target).astype(np.int64)
    ta = np.asarray(target_adv).astype(np.int64)
    f0 = np.asarray(feat_pooled, dtype=np.float32)
    f1 = np.asarray(feat_pooled_masked, dtype=np.float32)
    feats = np.concatenate([f0, f1], axis=0)  # [2B, D]
    iota = np.tile(np.arange(C, dtype=np.float16), (128, 1))

    in_maps = []
    for cc in range(NCORES):
        # GLOBAL roll: preserves mod-8192 circulant distances, so the
        # half-open band covers each unordered pair exactly once fleet-wide.
        order = (np.arange(N2) + cc * RB) % N2
        ftc = np.ascontiguousarray(
            feats[order].T.astype(ml_dtypes.bfloat16))  # [D, 2B]
        tgc = tg[cc * RB:(cc + 1) * RB].reshape(NT, 128).T
        tac = ta[cc * RB:(cc + 1) * RB].reshape(NT, 128).T
        fp = np.concatenate([iota, tgc.astype(np.float16),
                             tac.astype(np.float16)], axis=1)
        in_maps.append({
            "oma": np.ascontiguousarray(
                oma[4 * cc:4 * cc + 4].reshape(RB, 3 * C)),
            "ft": ftc,
            "fp": np.ascontiguousarray(fp),
        })
    return in_maps


def _combine(results):
    r = np.zeros(8, dtype=np.float64)
    rs = np.zeros(N2, dtype=np.float64)
    for cc, rr in enumerate(results):
        r += rr["res"].reshape(-1).astype(np.float64)
        rsp = rr["rsp"].astype(np.float64)        # [128, NQ*NSLOT]
        cspv = rr["csp"].astype(np.float64)       # [128, NBLK]
        rolled = cspv.T.reshape(-1).copy()        # rolled col 128*blk+p
        for k, Bq in enumerate(QBLKS):
            rows = slice(128 * Bq, 128 * Bq + 128)
            rolled[rows] += rsp[:, NSLOT * k:NSLOT * (k + 1)].sum(axis=1)
        order = (np.arange(N2) + cc * RB) % N2
        rs[order] += rolled
    kl_mean = r[0] / (B * C)
    ce_mean = r[1] / B
    adv_mean = r[2] / B
    foc_mean = r[3] / B
    pos_sum = 2.0 * r[4] / NCE_TEMP          # sum of positive logits, all rows
    lse = np.log(rs)
    nce_mean = (lse.sum() - pos_sum) / N2
    loss = (KL_INTERP * KL_TEMP * KL_TEMP) * kl_mean \
        + (1.0 - KL_INTERP) * ce_mean + nce_mean + foc_mean + adv_mean
    return np.asarray([loss], dtype=np.float32)


def kernel(**inputs):
    in_maps = _prep_inputs(**inputs)
    out = run_bass_kernel_spmd(_get_nc(), in_maps,
                               core_ids=list(range(NCORES)))
    return _combine(out.results)


if __name__ == "__main__":
    rng = np.random.default_rng(0)
    ins = {
        "output": rng.standard_normal((B, C), dtype=np.float32),
        "target": rng.integers(0, C, size=(B,)),
        "master_net_pred": rng.standard_normal((B, C), dtype=np.float32),
        "feat_pooled": rng.standard_normal((B, D), dtype=np.float32),
        "feat_pooled_masked": rng.standard_normal((B, D), dtype=np.float32),
        "output_adv": rng.standard_normal((B, C), dtype=np.float32),
        "target_adv": rng.integers(0, C, size=(B,)),
    }
    print(kernel(**ins))
